# revision 18
# baseline (speedup 1.0000x reference)
"""Trainium2 Bass kernel for nn_Classifier (gather + 2-layer MLP).

Reference computation (per batch b of 512, per position m of 128):
    tx      = X[b, idx_M[b, m], :]                            # [256]
    concat  = [tx, molvec[b]]                                 # [512]
    h       = relu(W1 @ concat + b1)                          # [512]
    out     = W2 @ h + b2                                     # [512]

Sharding: data-parallel over batch across 8 NeuronCores (64 batches/core).

Per-core kernel plan:
  - dma_gather (SWDGE) pulls the 8192 indexed rows of X (1KB each) from HBM
    into SBUF laid out [m=partition, batch=free-chunk, feat].
  - W1 is split into W1a (acts on gathered x) and W1b (acts on molvec).
    The molvec half is computed once per batch (64 rows instead of 8192):
    hmvT[b, :] = W1b @ molvec[b] + b1, and injected into layer-1 PSUM with a
    one-hot selector matmul (contracting over the 64 batches), since layer-1
    output is kept transposed ([IN_DIM, rows]).
  - Layer 1: PE matmuls with lhsT=W1a^T tiles (stationary), rhs = x^T tiles
    (from PE transposes), accumulating onto the bias in PSUM; one big ReLU
    per supertile on the PSUM->SBUF copy (ScalarE).
  - Layer 2: lhsT = hT tiles directly (no transposes needed), rhs = W2^T,
    output [rows, 512] row-major in PSUM; VectorE adds b2 (broadcast from
    host) on the PSUM->SBUF copy; contiguous 256KB DMA stores.
"""

import numpy as np

B, N, M = 512, 1024, 128
OUT_DIM, MOLVEC_DIM, IN_DIM = 256, 256, 512
NCORES = 8
BC = B // NCORES            # 64 batches per core
R = BC * M                  # 8192 output rows per core
HALF_B = 32                 # batches per half-table (int16 index range)
HALF_ROWS = HALF_B * N      # 32768 rows

# batches per dma_gather op; first ops smaller so compute starts earlier
GATHER_PLAN = (2, 2, 4, 8, 8, 8, 8, 8, 8, 8)
assert sum(GATHER_PLAN) == BC
GATHER_OFF = tuple(int(x) for x in np.cumsum((0,) + GATHER_PLAN[:-1]))

# molvec bias via PE one-hot matmul (True) vs per-batch ACT bias (False)
BIAS_VIA_PE = True

# Matmul input dtype for the MLP matmuls: "float32r" streams fp32 through the
# PE at full rate when the moving dim is >= 256 (vs 1/4 rate for plain fp32).
# Operands must be rounded to fp32r by a compute-engine write first; the
# rounding rides existing copies (weights: one-time staging copy; xgT: the
# PSUM->SBUF copy; hT: the ReLU activation write). "bfloat16" also works as
# a faster/lower-precision fallback; "float32" is exact but 4x slower on PE.
MM_DT_NAME = "float32r"

_CACHE = {}
LAST_EXEC_NS = None
LAST_RESULTS = None


def _build_nc(reps=1):
    import concourse.bacc as bacc
    import concourse.mybir as mybir
    import concourse.tile as tile
    from concourse.masks import make_identity
    from contextlib import ExitStack

    f32 = mybir.dt.float32
    i16 = mybir.dt.int16
    mm_dt = getattr(mybir.dt, MM_DT_NAME)
    AF = mybir.ActivationFunctionType

    nc = bacc.Bacc("TRN2", target_bir_lowering=False, debug=False,
                   num_devices=NCORES)

    x_d = nc.dram_tensor("x", [BC * N, OUT_DIM], f32, kind="ExternalInput").ap()
    gidx_d = nc.dram_tensor("gidx", [128, BC * M // 16], i16,
                            kind="ExternalInput").ap()
    mvt_d = nc.dram_tensor("mvt", [MOLVEC_DIM, BC], f32, kind="ExternalInput").ap()
    w1at_d = nc.dram_tensor("w1at", [OUT_DIM, IN_DIM], f32, kind="ExternalInput").ap()
    w1bt_d = nc.dram_tensor("w1bt", [MOLVEC_DIM, IN_DIM], f32, kind="ExternalInput").ap()
    w2t_d = nc.dram_tensor("w2t", [IN_DIM, IN_DIM], f32, kind="ExternalInput").ap()
    b1r_d = nc.dram_tensor("b1r", [1, IN_DIM], f32, kind="ExternalInput").ap()
    b2b_d = nc.dram_tensor("b2b", [128, IN_DIM], f32, kind="ExternalInput").ap()
    oh_d = nc.dram_tensor("oh", [BC, (BC // 2) * 256], f32, kind="ExternalInput").ap()
    out_d = nc.dram_tensor("out", [R, IN_DIM], f32, kind="ExternalOutput").ap()

    with tile.TileContext(nc) as tc, ExitStack() as ctx:
        const = ctx.enter_context(tc.tile_pool(name="const", bufs=1))
        xgp = ctx.enter_context(tc.tile_pool(name="xg", bufs=1))

        # --- persistent SBUF tensors ---
        gidx_sb = const.tile([128, BC * M // 16], i16, tag="gidx")
        w1at_sb = const.tile([128, 2 * IN_DIM], f32, tag="w1at")
        w1bt_sb = const.tile([128, 2 * IN_DIM], f32, tag="w1bt")
        w2t_sb = const.tile([128, 4 * IN_DIM], f32, tag="w2t")
        mvt_sb = const.tile([128, 2 * BC], f32, tag="mvt")
        b1r_sb = const.tile([1, IN_DIM], f32, tag="b1r")
        b2b_sb = const.tile([128, IN_DIM], f32, tag="b2b")
        ident = const.tile([128, 128], f32, tag="ident")
        ones_sb = const.tile([1, BC], f32, tag="ones")
        hmvt_sb = const.tile([BC, IN_DIM], mm_dt, tag="hmvt")
        oh_r = const.tile([BC, (BC // 2) * 256], mm_dt, tag="ohr")

        # --- small loads first: weights for hmv + layer 1 start fast ---
        nc.sync.dma_start(gidx_sb[:], gidx_d[:])
        for k in range(2):
            nc.sync.dma_start(mvt_sb[:, k * BC:(k + 1) * BC],
                              mvt_d[k * 128:(k + 1) * 128, :])
            nc.sync.dma_start(w1bt_sb[:, k * IN_DIM:(k + 1) * IN_DIM],
                              w1bt_d[k * 128:(k + 1) * 128, :])
            nc.sync.dma_start(w1at_sb[:, k * IN_DIM:(k + 1) * IN_DIM],
                              w1at_d[k * 128:(k + 1) * 128, :])
        nc.sync.dma_start(b1r_sb[:], b1r_d[:])
        nc.gpsimd.memset(ones_sb[:], 1.0)
        make_identity(nc, ident[:])

        # --- gathers: the long pole; first ops small so compute starts early
        def emit_gathers():
            tiles = []
            for q, nb in enumerate(GATHER_PLAN):
                off = GATHER_OFF[q]
                t = xgp.tile([128, nb * OUT_DIM], f32, tag=f"xg{q}")
                tiles.append(t)
                half = off // HALF_B
                nc.gpsimd.dma_gather(
                    out_ap=t[:].rearrange("p (g e) -> p g e", e=OUT_DIM),
                    in_ap=x_d[half * HALF_ROWS:(half + 1) * HALF_ROWS, :],
                    idxs_ap=gidx_sb[:, off * (M // 16):(off + nb) * (M // 16)],
                    num_idxs=nb * M,
                    num_idxs_reg=nb * M,
                    elem_size=OUT_DIM,
                )
            return tiles

        xg_tiles = emit_gathers()

        # --- remaining constants ---
        for k in range(4):
            nc.sync.dma_start(w2t_sb[:, k * IN_DIM:(k + 1) * IN_DIM],
                              w2t_d[k * 128:(k + 1) * 128, :])
        nc.sync.dma_start(b2b_sb[:], b2b_d[:])

        # --- molvec half of layer 1: hmvT[b, i] = (W1b @ molvec[b])[i] + b1[i]
        with tc.tile_pool(name="hmvp", bufs=1, space="PSUM") as hmvp:
            hmv_ps = hmvp.tile([BC, IN_DIM], mybir.dt.float32, tag="hmvps")
            for k2 in range(2):
                nc.tensor.matmul(
                    out=hmv_ps[:],
                    lhsT=mvt_sb[:, k2 * BC:(k2 + 1) * BC],
                    rhs=w1bt_sb[:, k2 * IN_DIM:(k2 + 1) * IN_DIM],
                    start=(k2 == 0), stop=False,
                )
            nc.tensor.matmul(out=hmv_ps[:], lhsT=ones_sb[:], rhs=b1r_sb[:],
                             start=False, stop=True)
            nc.vector.tensor_copy(out=hmvt_sb[:], in_=hmv_ps[:])

        if mm_dt != f32:
            with tc.tile_pool(name="ohstage", bufs=1) as ohs:
                oh_f32 = ohs.tile([BC, (BC // 2) * 256], f32, tag="ohf")
                nc.sync.dma_start(oh_f32[:], oh_d[:])
                nc.vector.tensor_copy(out=oh_r[:], in_=oh_f32[:])
        else:
            nc.sync.dma_start(oh_r[:], oh_d[:])

        # round the main-loop weights to the matmul dtype (one-time)
        if mm_dt != f32:
            w1at_r = const.tile([128, 2 * IN_DIM], mm_dt, tag="w1atr")
            w2t_r = const.tile([128, 4 * IN_DIM], mm_dt, tag="w2tr")
            nc.vector.tensor_copy(out=w1at_r[:], in_=w1at_sb[:])
            nc.vector.tensor_copy(out=w2t_r[:], in_=w2t_sb[:])
        else:
            w1at_r, w2t_r = w1at_sb, w2t_sb

        def batch_src(b):
            """(gather tile, within-tile offset) for batch b."""
            for q, nb in enumerate(GATHER_PLAN):
                if GATHER_OFF[q] <= b < GATHER_OFF[q] + nb:
                    return xg_tiles[q], b - GATHER_OFF[q]
            raise AssertionError(b)

        # --- main loop: 32 supertiles of 256 rows (2 batches) each ---
        trp = ctx.enter_context(tc.tile_pool(name="trp", bufs=2, space="PSUM"))
        htp = ctx.enter_context(tc.tile_pool(name="htp", bufs=2, space="PSUM"))
        outp = ctx.enter_context(tc.tile_pool(name="outp", bufs=2, space="PSUM"))
        xgtp = ctx.enter_context(tc.tile_pool(name="xgtp", bufs=3))
        htsp = ctx.enter_context(tc.tile_pool(name="htsp", bufs=2))
        outsp = ctx.enter_context(tc.tile_pool(name="outsp", bufs=4))

        for rep in range(reps):
          if rep > 0:
            xg_tiles = emit_gathers()
          for st in range(BC // 2):
            # transpose gathered x for the two batches: xgT[c][:, j*128:...]
            tr_ps = trp.tile([128, 512], mybir.dt.float32, tag="tr")
            for j in range(2):
                b = 2 * st + j
                xt, g = batch_src(b)
                for c in range(2):
                    nc.tensor.transpose(
                        out=tr_ps[:, c * 256 + j * 128: c * 256 + (j + 1) * 128],
                        in_=xt[:, g * OUT_DIM + c * 128:
                               g * OUT_DIM + (c + 1) * 128],
                        identity=ident[:],
                    )
            xgt_sb = xgtp.tile([128, 512], mm_dt, tag="xgt")
            nc.vector.tensor_copy(out=xgt_sb[:], in_=tr_ps[:])

            # layer 1: hT[m-chunk][:, rows 0:256]; bias first, then 2 K-chunks
            ht_ps = htp.tile([128, 1024], mybir.dt.float32, tag="ht")
            for m in range(4):
                if BIAS_VIA_PE:
                    nc.tensor.matmul(
                        out=ht_ps[:, m * 256:(m + 1) * 256],
                        lhsT=hmvt_sb[:, m * 128:(m + 1) * 128],
                        rhs=oh_r[:, st * 256:(st + 1) * 256],
                        start=True, stop=False,
                    )
                for c2 in range(2):
                    nc.tensor.matmul(
                        out=ht_ps[:, m * 256:(m + 1) * 256],
                        lhsT=w1at_r[:, c2 * IN_DIM + m * 128:
                                    c2 * IN_DIM + (m + 1) * 128],
                        rhs=xgt_sb[:, c2 * 256:(c2 + 1) * 256],
                        start=(not BIAS_VIA_PE and c2 == 0), stop=(c2 == 1),
                    )

            # relu -> SBUF (rounds to matmul dtype)
            ht_sb = htsp.tile([128, 1024], mm_dt, tag="hts")
            nc.scalar.activation(out=ht_sb[:], in_=ht_ps[:], func=AF.Relu)

            # layer 2 + b2 + store, per batch
            for j in range(2):
                b = 2 * st + j
                o_ps = outp.tile([128, IN_DIM], mybir.dt.float32, tag="o")
                for k in range(4):
                    nc.tensor.matmul(
                        out=o_ps[:],
                        lhsT=ht_sb[:, k * 256 + j * 128:
                                   k * 256 + (j + 1) * 128],
                        rhs=w2t_r[:, k * IN_DIM:(k + 1) * IN_DIM],
                        start=(k == 0), stop=(k == 3),
                    )
                o_sb = outsp.tile([128, IN_DIM], f32, tag="os")
                nc.vector.tensor_tensor(out=o_sb[:], in0=o_ps[:],
                                        in1=b2b_sb[:],
                                        op=mybir.AluOpType.add)
                nc.sync.dma_start(out_d[b * M:(b + 1) * M, :], o_sb[:])

    nc.compile()
    return nc


def get_nc(reps=1):
    key = ("nc", reps)
    if key not in _CACHE:
        _CACHE[key] = _build_nc(reps)
    return _CACHE[key]


def make_in_maps(X, molvec, idx_M):
    X = np.ascontiguousarray(np.asarray(X, dtype=np.float32))
    molvec = np.ascontiguousarray(np.asarray(molvec, dtype=np.float32))
    idx = np.asarray(idx_M)

    # one-hot selector: oh[b, st*256 + r] = 1 iff b == 2*st + r//128
    oh = np.zeros((BC, (BC // 2) * 256), dtype=np.float32)
    for st in range(BC // 2):
        oh[2 * st, st * 256:st * 256 + 128] = 1.0
        oh[2 * st + 1, st * 256 + 128:(st + 1) * 256] = 1.0

    in_maps = []
    for c in range(NCORES):
        xs = X[c * BC:(c + 1) * BC].reshape(BC * N, OUT_DIM)
        ic = idx[c * BC:(c + 1) * BC].astype(np.int64)      # [BC, M]
        loc = ((np.arange(BC)[:, None] % HALF_B) * N + ic)  # local row in half
        loc = loc.astype(np.int16)                          # max 32767, fits
        cols = []
        for q, nb in enumerate(GATHER_PLAN):
            off = GATHER_OFF[q]
            arr = loc[off:off + nb].reshape(-1)             # j = lb*128 + m
            wrapped = arr.reshape(-1, 16).T                 # idx j at [j%16, j//16]
            cols.append(np.tile(wrapped, (8, 1)))           # [128, nb*8]
        gidx = np.ascontiguousarray(np.concatenate(cols, axis=1))
        mvt = np.ascontiguousarray(molvec[c * BC:(c + 1) * BC].T)
        in_maps.append({"x": xs, "gidx": gidx, "mvt": mvt, "oh": oh})
    return in_maps


def kernel(X, molvec, idx_M, W1, b1, W2, b2, trace=False):
    global LAST_EXEC_NS, LAST_RESULTS
    from concourse.bass_utils import run_bass_kernel_spmd

    W1 = np.asarray(W1, dtype=np.float32)
    W2 = np.asarray(W2, dtype=np.float32)
    b1 = np.asarray(b1, dtype=np.float32)
    b2 = np.asarray(b2, dtype=np.float32)

    w1at = np.ascontiguousarray(W1[:, :OUT_DIM].T)            # [256, 512]
    w1bt = np.ascontiguousarray(W1[:, OUT_DIM:].T)            # [256, 512]
    w2t = np.ascontiguousarray(W2.T)                          # [512, 512]
    b1r = np.ascontiguousarray(b1.reshape(1, IN_DIM))
    b2b = np.ascontiguousarray(np.broadcast_to(b2, (128, IN_DIM)))

    in_maps = make_in_maps(X, molvec, idx_M)
    for im in in_maps:
        im.update({"w1at": w1at, "w1bt": w1bt, "w2t": w2t,
                   "b1r": b1r, "b2b": b2b})

    nc = get_nc()
    res = run_bass_kernel_spmd(nc, in_maps, list(range(NCORES)), trace=trace)
    LAST_EXEC_NS = res.exec_time_ns
    LAST_RESULTS = res
    out = np.concatenate([res.results[c]["out"] for c in range(NCORES)], axis=0)
    return out
